# revision 34
# baseline (speedup 1.0000x reference)
"""CrossModalAttention fused Bass/Tile kernel for Trainium2 (8 NeuronCores).

Math (per batch b):
    pooled = mean_w x_skel[b]                      # [Cs, Ws]
    k  = Wk @ pooled + bk                          # [Ci, Ws]
    q  = Wq @ x_rgb[b] + bq                        # (never materialized)
    energy = q^T k = x_rgb^T (Wq^T k) + 1 (bq^T k) # [HW, Ws]  <- low-rank trick
    att = softmax(energy, axis=-1)
    v  = Wv @ pooled + bv
    out = gamma * (v @ att^T) + x_rgb

Weight-only host folds (exact algebra, fp64 accumulation):
    Wkq = (Wk/25)^T Wq   [Cs, Cr]   so  Wq^T k = Wkq^T pooled_sum + bkq
    bkq = Wq^T bk        [Cr]
    u   = Wk^T bq / 25   [Cs]       so  bq^T k = u^T pooled_sum + cbb
    cbb = bq . bk        scalar
    WvT = gamma (Wv/25)^T [Cs, Cr]  (gamma folded into v)
    gbv = gamma bv       [Cr]       (row 25 of the vT stationary)

fp8 (e4m3) for the attention path, with power-of-2 pre-scales because the
raw folded weights sit deep in fp8's subnormal range (Wkq std ~2e-3,
WvT std ~5e-4):
    HBM holds Wkq*S1, bkq*S1, u*S3, cbb*S3, WvT*S2, gbv*S2 in fp8/bf16 and
    x_skel in fp8 -- 0.8 MB/core less HBM traffic and earlier kq start.
    Descaling is free:
      * kq_sb keeps the S1 scale in bf16; exp() runs with scale=1/S1.
      * be = (u^T pooled + cbb) descales via the ACT copy's scale=1/S3.
      * the softmax 'ones' column is memset to S2, so r = 1/(S2*s) and
        attT = att/S2 exactly cancels the S2 in vT; attT row 25 = 1/S2.
    x_rgb and out stay bf16 (the +x residual dominates the error budget;
    fp8 x would miss the 2e-2 tolerance).  Validated: rel_err ~1.0e-2.

Schedule (fixes for the 55us baseline, whose trace showed the PE
re-throttled to 1.2 GHz for the whole output phase and the out-DMA only
started at 34us):
  * out PSUM tiles are [128, HW] 2-bank tiles (matmuls write per-bank NT
    slices) in a 3-deep ring (the energy tiles share the psS ring to free
    the banks); each drain is ONE [128, HW] op, spread over the only
    PSUM-capable engines -- DVE tensor_add(psum+x) for rts 1,3,5,7; ACT
    copy with the +x residual pre-accumulated on the PE via an identity
    matmul for 'a' rts; ACT copy + GPSIMD in-place SBUF add for 'p' rts
    (GPSIMD cannot read PSUM on TRN2). b1 drains its 'p' rts early and
    its tail rts on fast engines so the last DMA chunks aren't gated on
    a ~2us gpsimd add.
  * per-batch pipelining: out_phase(0) + its per-rt ~200KB DMA chunks run
    interleaved with energy/softmax(1); out DMAs ride the SP HWDGE ring
    (idle after the input stream) so the ACT queue stays clear for
    exp/drain work.
  * kq is built as two independent half tiles (dense matmuls, one ACT
    copy each -- a single tile caused a PE<->ACT write-after-read
    ping-pong). The softmax sum matmul uses an all-ones*S2 [25,25]
    stationary so every output row holds the column sum: one DVE
    fast-reciprocal on [25,nn] then directly yields the broadcast R
    (no bf16 cast, no PE row-broadcast), cutting the serial chain to
    exp -> sum -> recip -> mul.
  * b0's x arrives in 2-t quarter chunks so energy kt's unblock
    progressively, and heat matmuls bridge the x00/x01 DMA waits --
    otherwise the idle re-throttles the HAM clock gate (observed
    ~3.4us-quantized grants) right before the energy matmuls.
  * ~24 full-array warm-up matmuls run in the (still idle) psO ring
    while inputs stream in, sized so the 2.4 GHz grant starts just
    before kq and covers the energy/softmax/out(0) phases.

Sharding: pure data-parallel over batch B=16 -> 2 batches per NeuronCore.
"""

import os
import sys

for _p in ("/opt/trn_rl_repo", "/root/.axon_site/_ro/trn_rl_repo"):
    if os.path.isdir(_p) and _p not in sys.path:
        sys.path.insert(0, _p)

import ml_dtypes
import numpy as np

import concourse.bass as bass  # noqa: F401
import concourse.mybir as mybir
import concourse.tile as tile
from concourse import bacc
from concourse.bass_utils import run_bass_kernel_spmd
from concourse.masks import make_identity

B, Cr, H, W = 16, 1024, 28, 28
Cs, Hs, Ws = 256, 25, 25
Ci = 512
HW = H * W  # 784
SK = Hs * Ws  # 625
N_CORES = 8
BPC = B // N_CORES  # batches per core = 2
WA = Ws + 1  # 26 rows: 0..24 att, 25 = ones/bias row
NT = (512, 272)  # free-dim tiling of HW=784, PSUM-bank aligned
S1 = 2.0**9  # Wkq/bkq fp8 pre-scale (descaled by exp's ACT scale)
S2 = 2.0**11  # WvT/gbv fp8 pre-scale (descaled via attT = att/S2)
S3 = 2.0**11  # u/cbb fp8 pre-scale (descaled by be's ACT-copy scale)
N_WARM = 16  # dummy PE matmuls: flip + hold the HAM clock gate at 2.4GHz
# drain engine per rt: 'v'=DVE tensor_add(psum+x); 'a'=ACT copy with the
# residual pre-added on the PE via identity matmul; 'p'=ACT copy then
# GPSIMD in-place SBUF add of x (GPSIMD cannot access PSUM on TRN2).
# b1 takes its slow pool-drains early and fast drains on the tail rts so
# the final DMA chunks aren't gated on a ~2us gpsimd add.
DRAIN = (
    ("a", "v", "p", "v", "a", "v", "p", "v"),
    ("v", "v", "p", "v", "a", "v", "a", "v"),
)
FP = mybir.dt.float32
BF = mybir.dt.bfloat16
F8 = mybir.dt.float8e4
AX = mybir.AxisListType
AF = mybir.ActivationFunctionType
ALU = mybir.AluOpType
BF_NP = ml_dtypes.bfloat16
F8_NP = ml_dtypes.float8_e4m3fn


def _nt_slices():
    off = 0
    for n in NT:
        yield off, n
        off += n


def _build():
    nc = bacc.Bacc(None, target_bir_lowering=False)

    xr_d = nc.dram_tensor("xr", [128, BPC * 8 * HW], BF, kind="ExternalInput")
    xs_d = nc.dram_tensor("xs", [128, 2 * BPC * SK], F8, kind="ExternalInput")
    Wkq_d = nc.dram_tensor("Wkq", [128, 2 * Cr], F8, kind="ExternalInput")
    WvT_d = nc.dram_tensor("WvT", [128, 2 * Cr], F8, kind="ExternalInput")
    bks_d = nc.dram_tensor("bks", [8, 528], F8, kind="ExternalInput")
    # smb cols: 0..1 = u*S3 (ct halves), 2 = cbb*S3 (replicated), 3 = pad
    smb_d = nc.dram_tensor("smb", [128, 4], F8, kind="ExternalInput")
    gbv_d = nc.dram_tensor("gbv", [1, Cr], BF, kind="ExternalInput")
    out_d = nc.dram_tensor("out", [128, BPC * 8 * HW], BF, kind="ExternalOutput")

    xr_dv = xr_d.rearrange("p (b t n) -> p b t n", b=BPC, t=8)
    out_dv = out_d.rearrange("p (b t n) -> p b t n", b=BPC, t=8)

    with tile.TileContext(nc) as tc:
        with (
            nc.allow_low_precision(reason="bf16/fp8 pipeline (tolerance 2e-2)"),
            tc.tile_pool(name="const", bufs=1) as const,
            tc.tile_pool(name="wt", bufs=1) as wt,
            tc.tile_pool(name="xp", bufs=2) as xp,
            tc.tile_pool(name="work", bufs=2) as work,
            tc.tile_pool(name="outp", bufs=2) as outp,
            tc.tile_pool(name="psS", bufs=2, space="PSUM") as psS,
            tc.tile_pool(name="psO", bufs=3, space="PSUM") as psO,
        ):
            # ---- consts first on gpsimd (gate the PE warm-up matmuls)
            warm_src = const.tile([128, 512], BF, tag="warm_src")
            nc.gpsimd.memset(warm_src, 1.0)

            # ---- input DMAs. sync(SP ring): x_skel then x_rgb; weights on
            # the gpsimd SWDGE queues; WvT/gbv on the ACT HWDGE ring (which
            # later carries the out-DMAs).
            xs_sb = wt.tile([128, 2, BPC, SK], F8, tag="xs")
            xs_dv = xs_d.rearrange("p (c b j) -> p c b j", c=2, b=BPC)
            for ct in range(2):
                nc.sync.dma_start(xs_sb[:, ct], xs_dv[:, ct])
            Wkq_sb = wt.tile([128, 2, Cr], F8, tag="wkq")
            nc.gpsimd.dma_start(Wkq_sb, Wkq_d.rearrange("p (c r) -> p c r", c=2))
            bks_sb = wt.tile([8, 528], F8, tag="bks")
            nc.gpsimd.dma_start(bks_sb, bks_d[:])
            smb_sb = wt.tile([128, 4], F8, tag="smb")
            nc.gpsimd.dma_start(smb_sb, smb_d[:])
            x_sbs = []  # [b][half] -> [128, 4, HW]
            for b in range(BPC):
                halves = []
                for h in range(2):
                    x_sb = xp.tile([128, 4, HW], BF, tag=f"x{h}", name=f"x{b}_{h}")
                    for q in range(2):
                        nc.sync.dma_start(
                            x_sb[:, q * 2 : (q + 1) * 2],
                            xr_dv[:, b, h * 4 + q * 2 : h * 4 + (q + 1) * 2, :],
                        )
                    halves.append(x_sb)
                x_sbs.append(halves)
            WvT_sb = wt.tile([128, 2, Cr], F8, tag="wvt")
            nc.gpsimd.dma_start(WvT_sb, WvT_d.rearrange("p (c r) -> p c r", c=2))

            # vT stationaries [WA, Cr]: row 25 = S2*gamma*bv lands via DMA,
            # rows 0..24 come from the v matmul PSUM copies below.
            vT_sbs = []
            for b in range(BPC):
                vT = wt.tile([WA, Cr], BF, tag=f"vT{b}", name=f"vT{b}")
                nc.gpsimd.dma_start(vT[25:26, :], gbv_d[:])
                vT_sbs.append(vT)

            # remaining consts on gpsimd, after the DMA issues
            ones_r8 = const.tile([1, 64], F8, tag="ones_r8")
            nc.gpsimd.memset(ones_r8, 1.0)
            # all-ones*S2 [25,25]: the sum matmul then writes the column
            # sum to EVERY output row, so one reciprocal yields the already-
            # broadcast R (no bf16 cast, no PE row-broadcast matmul)
            ones_sq = const.tile([Ws, Ws], BF, tag="ones_sq")
            nc.gpsimd.memset(ones_sq, S2)
            ident = const.tile([128, 128], BF, tag="ident")
            make_identity(nc, ident)

            # per-batch softmax work tiles; attT row 25 = exact 1/S2
            E_sbs, attTs, r25s = [], [], []
            for b in range(BPC):
                E_sb = work.tile([Ws, HW], BF, tag="E", name=f"E{b}")
                attT = work.tile([WA, HW], BF, tag="attT", name=f"attT{b}")
                # row 25 must be exactly 1/S2 (bias row of the out matmul);
                # partition-25 slices aren't addressable by compute engines,
                # so memset the whole tile - rows 0..24 get overwritten.
                nc.vector.memset(attT, 1.0 / S2)
                r25 = work.tile([Ws, HW], FP, tag="r25", name=f"r25_{b}")
                E_sbs.append(E_sb)
                attTs.append(attT)
                r25s.append(r25)

            # ---- PE warm-up in the (still idle) psO ring. Gated on the
            # xs ct0 DMA (xs_sb is the moving operand): the HAM clock gate
            # allows only ~20.5us at 2.4 GHz per kernel, so the flip is
            # delayed until real work starts and the window covers the
            # energy+out phases instead of being burned on idle bridging.
            for i in range(N_WARM):
                wp = psO.tile([128, 512], FP, tag="op", name=f"warm{i}")
                nc.tensor.matmul(
                    wp,
                    warm_src[:, 0:128],
                    warm_src,
                    start=True,
                    stop=True,
                )

            # ---- pooled_sum [128(cs%), ct, b, Ws] fp8 (1/25 in weights)
            pooled = wt.tile([128, 2, BPC, Ws], F8, tag="pooled")
            # per-(ct,b) reduces: each starts as soon as its xs half lands
            for ct in range(2):
                for b in range(BPC):
                    nc.vector.reduce_sum(
                        pooled[:, ct, b],
                        xs_sb[:, ct, b].rearrange("p (h w) -> p h w", w=Ws),
                        axis=AX.X,
                    )

            # ---- kq [128(c%), kt, b*Ws+h] = S1*(Wkq^T pooled + bkq), one
            # PSUM tile: bias via block-selector matmul, two ct-major
            # accumulation passes, one copy out (kept S1-scaled in bf16).
            kq_sb = wt.tile([128, 8, BPC * Ws], BF, tag="kq")
            # two independent half tiles: dense matmuls per half with no
            # write-after-read hazard against the other half's ACT copy
            for h in range(2):
                kqp = psS.tile(
                    [128, 4, BPC * Ws], FP, tag="ps", name=f"kqp{h}"
                )
                nc.tensor.matmul(
                    kqp,
                    bks_sb[:, 0:128],
                    bks_sb[:, 128 + h * 200 : 328 + h * 200],
                    start=True,
                    stop=False,
                )
                # ct-major: all ct0 matmuls run as soon as the ct0 reduce
                # lands instead of ping-ponging on the later ct1 reduce
                for ct in range(2):
                    if h == 0 and ct == 1:
                        # bridge the pooled-ct1 wait in the PE pipe
                        for _ in range(2):
                            nc.tensor.ldweights(warm_src[:, 0:128])
                    for kt in range(4):
                        nc.tensor.matmul(
                            kqp[:, kt],
                            Wkq_sb[:, ct, (h * 4 + kt) * 128 : (h * 4 + kt + 1) * 128],
                            pooled[:, ct],
                            start=False,
                            stop=(ct == 1 and kt == 3),
                            skip_group_check=True,
                        )
                nc.scalar.copy(kq_sb[:, h * 4 : h * 4 + 4], kqp)

            # ---- helpers ----------------------------------------------
            heatn = [0]

            def heat_mm(n):
                # full-array dummy matmuls into the psO ring (registers as
                # PE activity for the HAM gate, unlike LDWEIGHTS)
                for _ in range(n):
                    heatn[0] += 1
                    hp = psO.tile(
                        [128, 512], FP, tag="op", name=f"heat{heatn[0]}"
                    )
                    nc.tensor.matmul(
                        hp, warm_src[:, 0:128], warm_src, start=True, stop=True
                    )

            def heat(n):
                # dummy weight loads: cheap PE-pipeline filler with no PSUM
                # bank and no ring dependencies at all
                for _ in range(n):
                    nc.tensor.ldweights(warm_src[:, 0:128])

            def energy_mms(b, eTs, kts):
                for kt in kts:
                    for (nof, nn), eT in zip(_nt_slices(), eTs):
                        nc.tensor.matmul(
                            eT[0:Ws, 0:nn],
                            kq_sb[:, kt, b * Ws : (b + 1) * Ws],
                            x_sbs[b][kt // 4][:, kt % 4, nof : nof + nn],
                            start=(kt == 0),
                            stop=(kt == 7),
                            skip_group_check=True,
                        )

            def be_mms(b):
                # be[s, b] = S3*(u^T pooled + cbb); descaled by the copy
                ps = psS.tile([Ws, 1], FP, tag="ps", name=f"be{b}")
                nc.tensor.matmul(
                    ps, ones_r8[0:1, 0:Ws], smb_sb[0:1, 2:3], start=True, stop=False
                )
                for ct in range(2):
                    nc.tensor.matmul(
                        ps,
                        pooled[:, ct, b],
                        smb_sb[:, ct : ct + 1],
                        start=False,
                        stop=(ct == 1),
                    )
                nc.scalar.activation(
                    be_sb[:, b : b + 1], ps, func=AF.Copy, scale=1.0 / S3
                )

            def v_mms(b):
                # vT rows 0..24 = S2 * gamma * v^T (S2, gamma folded in
                # WvT). Emitted late: fills the PE during softmax(0)'s
                # recip/mul instead of delaying energy(0).
                for nof in (0, 512):
                    ps2 = psS.tile([Ws, 512], FP, tag="ps", name=f"v{b}_{nof}")
                    for ct in range(2):
                        nc.tensor.matmul(
                            ps2,
                            pooled[:, ct, b],
                            WvT_sb[:, ct, nof : nof + 512],
                            start=(ct == 0),
                            stop=(ct == 1),
                        )
                    nc.scalar.copy(vT_sbs[b][0:Ws, nof : nof + 512], ps2)

            def sm_exp(b, eTs):
                # E = exp(eT/S1 + be) -- descales the kq-side S1
                for (nof, nn), eT in zip(_nt_slices(), eTs):
                    nc.scalar.activation(
                        E_sbs[b][:, nof : nof + nn],
                        eT[0:Ws, 0:nn],
                        func=AF.Exp,
                        bias=be_sb[:, b : b + 1],
                        scale=1.0 / S1,
                    )

            def sm_sum(b):
                # every row of srow25 = S2 * sum_s E (all-ones stationary)
                srows = []
                for nof, nn in _nt_slices():
                    srow = psS.tile([Ws, nn], FP, tag="ps", name=f"s{b}_{nof}")
                    nc.tensor.matmul(
                        srow,
                        ones_sq,
                        E_sbs[b][:, nof : nof + nn],
                        start=True,
                        stop=True,
                    )
                    srows.append(srow)
                return srows

            def sm_recip(b, srows):
                # r25 = 1/(S2*s), already broadcast across the 25 rows
                for (nof, nn), srow in zip(_nt_slices(), srows):
                    nc.vector.reciprocal_approx_fast(
                        r25s[b][:, nof : nof + nn], srow
                    )

            def sm_mul(b):
                # attT = E * r25 = att/S2 (cancels vT's S2; row 25 = 1/S2)
                for nof, nn in _nt_slices():
                    nc.vector.tensor_mul(
                        attTs[b][0:Ws, nof : nof + nn],
                        E_sbs[b][:, nof : nof + nn],
                        r25s[b][:, nof : nof + nn],
                    )

            def out_rt(b, o_sbs, rt):
                # one [128, HW] channel block: out = vT^T attT (+x); the
                # matmuls write per-bank NT slices of one 2-bank PSUM tile
                # so the drain is a single [128, HW] op
                eng = DRAIN[b][rt]
                xin = x_sbs[b][rt // 4][:, rt % 4, :]
                dst = o_sbs[rt // 4][:, rt % 4, :]
                ps = psO.tile([128, HW], FP, tag="op", name=f"op{b}_{rt}")
                for nof, nn in _nt_slices():
                    nc.tensor.matmul(
                        ps[:, nof : nof + nn],
                        vT_sbs[b][:, rt * 128 : (rt + 1) * 128],
                        attTs[b][:, nof : nof + nn],
                        start=True,
                        stop=(eng != "a"),
                        skip_group_check=True,
                    )
                    if eng == "a":
                        # accumulate the residual on the PE (identity
                        # stationary streams x); ACT then just copies
                        nc.tensor.matmul(
                            ps[:, nof : nof + nn],
                            ident,
                            xin[:, nof : nof + nn],
                            start=False,
                            stop=True,
                            skip_group_check=True,
                        )
                if eng == "v":
                    nc.vector.tensor_add(dst, ps, xin)
                else:
                    nc.scalar.copy(dst, ps)
                    if eng == "p":
                        nc.gpsimd.tensor_add(dst, dst, xin)

            def out_dma(b, o_sbs, rt):
                # per-rt chunk (~200KB) on the SP HWDGE ring, which is idle
                # once the inputs are streamed (keeps ACT clean); fine
                # chunks smooth the out stream and shorten the tail
                nc.sync.dma_start(
                    out_dv[:, b, rt : rt + 1, :],
                    o_sbs[rt // 4][:, rt % 4 : rt % 4 + 1, :],
                )

            # ---- main pipeline ----------------------------------------
            be_sb = wt.tile([Ws, BPC], FP, tag="be")
            eT0 = [
                psS.tile([Ws, nn], FP, tag="ps", name=f"eT0_{i}")
                for i, (_, nn) in enumerate(_nt_slices())
            ]
            o0 = [
                outp.tile([128, 4, HW], BF, tag=f"o{h}", name=f"o0_{h}")
                for h in range(2)
            ]
            o1 = [
                outp.tile([128, 4, HW], BF, tag=f"o{h}", name=f"o1_{h}")
                for h in range(2)
            ]

            be_mms(0)
            be_mms(1)
            heat_mm(3)  # bridge a late x00 without going idle
            energy_mms(0, eT0, range(2))
            energy_mms(0, eT0, range(2, 4))
            heat_mm(6)  # bridge the x01 wait without going idle
            energy_mms(0, eT0, range(4, 6))
            energy_mms(0, eT0, range(6, 8))
            sm_exp(0, eT0)
            heat_mm(2)  # fill the PE while exp(0) runs on ACT
            srows0 = sm_sum(0)
            sm_recip(0, srows0)
            v_mms(0)  # PE fill during recip; vT0 copies land before out(0)
            sm_mul(0)
            eT1 = [
                psS.tile([Ws, nn], FP, tag="ps", name=f"eT1_{i}")
                for i, (_, nn) in enumerate(_nt_slices())
            ]
            energy_mms(1, eT1, range(4))
            energy_mms(1, eT1, range(4, 8))
            sm_exp(1, eT1)
            # out(0) starts as soon as attT0 lands; its DMA chunks overlap
            # the b1 input tail; softmax(1) interleaves into the early rts
            out_rt(0, o0, 0)
            out_dma(0, o0, 0)
            out_rt(0, o0, 1)
            out_dma(0, o0, 1)
            srows1 = sm_sum(1)
            sm_recip(1, srows1)
            sm_mul(1)
            v_mms(1)  # vT1 builds after softmax(1): not needed until out(1)
            out_rt(0, o0, 2)
            out_dma(0, o0, 2)
            out_rt(0, o0, 3)
            out_dma(0, o0, 3)
            out_rt(0, o0, 4)
            out_dma(0, o0, 4)
            out_rt(0, o0, 5)
            out_dma(0, o0, 5)
            out_rt(0, o0, 6)
            out_dma(0, o0, 6)
            out_rt(0, o0, 7)
            out_dma(0, o0, 7)
            for rt in range(8):
                out_rt(1, o1, rt)
                out_dma(1, o1, rt)

    nc.compile()
    return nc


_NC = None


def _get_nc():
    global _NC
    if _NC is None:
        _NC = _build()
    return _NC


def prepare_in_maps(x_rgb, x_skel, Wq, bq, Wk, bk, Wv, bv, gamma):
    """Host-side weight folds (weights only, exact fp64 algebra), dtype
    demotion to bf16/fp8, and SBUF-layout packing + per-core slicing."""
    xr = (
        np.asarray(x_rgb, np.float32)
        .reshape(B, 8, 128, HW)
        .transpose(2, 0, 1, 3)
        .astype(BF_NP)
    )  # [128(p), B, 8(t), HW]; channel c = t*128 + p
    xs = (
        np.asarray(x_skel, np.float32)
        .reshape(B, 2, 128, SK)
        .transpose(2, 1, 0, 3)
        .astype(F8_NP)
    )  # [128(p), 2(ct), B, SK]; cs = ct*128 + p
    Wq64 = np.asarray(Wq, np.float64)
    Wk64 = np.asarray(Wk, np.float64)
    Wv64 = np.asarray(Wv, np.float64)
    bq64 = np.asarray(bq, np.float64)
    bk64 = np.asarray(bk, np.float64)
    g64 = float(np.asarray(gamma, np.float64)[0])
    Wkq = S1 * (Wk64 / Ws).T @ Wq64  # [Cs, Cr], S1-scaled
    WvT = S2 * g64 * (Wv64 / Ws).T  # [Cs, Cr], S2-scaled
    bkq = S1 * Wq64.T @ bk64  # [Cr], S1-scaled
    u = S3 * Wk64.T @ bq64 / Ws  # [Cs], S3-scaled
    cbb = S3 * float(bq64 @ bk64)
    gbv = S2 * g64 * np.asarray(bv, np.float64)  # [Cr], S2-scaled

    Wkq_p = (
        np.ascontiguousarray(Wkq.reshape(2, 128, Cr).transpose(1, 0, 2))
        .reshape(128, 2 * Cr)
        .astype(F8_NP)
    )
    WvT_p = (
        np.ascontiguousarray(WvT.reshape(2, 128, Cr).transpose(1, 0, 2))
        .reshape(128, 2 * Cr)
        .astype(F8_NP)
    )
    bks = np.zeros((8, 528), np.float64)
    bks[:, 0:128] = bkq.reshape(8, 128)
    for k in range(8):
        bks[k, 128 + k * 50 : 128 + (k + 1) * 50] = 1.0
    bks_p = bks.astype(F8_NP)
    smb = np.zeros((128, 4), np.float64)
    smb[:, 0:2] = u.reshape(2, 128).T
    smb[:, 2] = cbb
    smb_p = smb.astype(F8_NP)
    gbv_p = gbv.reshape(1, Cr).astype(BF_NP)

    shared = {
        "Wkq": Wkq_p,
        "WvT": WvT_p,
        "bks": bks_p,
        "smb": smb_p,
        "gbv": gbv_p,
    }
    return [
        {
            "xr": np.ascontiguousarray(xr[:, c * BPC : (c + 1) * BPC]).reshape(
                128, BPC * 8 * HW
            ),
            "xs": np.ascontiguousarray(
                xs[:, :, c * BPC : (c + 1) * BPC]
            ).reshape(128, 2 * BPC * SK),
            **shared,
        }
        for c in range(N_CORES)
    ]


def kernel(x_rgb, x_skel, Wq, bq, Wk, bk, Wv, bv, gamma):
    nc = _get_nc()
    in_maps = prepare_in_maps(x_rgb, x_skel, Wq, bq, Wk, bk, Wv, bv, gamma)
    res = run_bass_kernel_spmd(nc, in_maps, core_ids=list(range(N_CORES)))
    outs = [
        np.asarray(r["out"])
        .reshape(128, BPC, 8, HW)
        .astype(np.float32)
        .transpose(1, 2, 0, 3)
        .reshape(BPC, Cr, H, W)
        for r in res.results
    ]
    return np.concatenate(outs, axis=0)
